# revision 1
# baseline (speedup 1.0000x reference)
"""Trainium2 Bass kernel for CrossHeadMultiHeadAttention.

Computation (per batch b, spatial site s=(h,w)):
  xr[s]   : [n=8 heads, d=64]
  LN over d, torch-Linear Q/K/V (64x64), cross-head attention within
  4 groups of head_dim/4=16 dims (8x8 score matrices per site/group),
  out-projection, residual.

Strategy: data-parallel over batch (16 batches -> 8 cores x 2).
Per-core pipeline per 128-site tile:
  - DMA x in [128=(2 heads x 64 d), sites] chunks (4 chunks = 8 heads)
  - PE-transpose to site-on-partition layout, bn_stats for LN stats
  - xhat = (x-mu)*rstd via ACT Identity(scale, bias) per head, bf16
  - PE-transpose back; QKV via bf16 matmuls w/ block-diagonal weights
    (LN gain/bias folded into weights host-side; proj bias via K=1 matmul)
  - attention core on DVE bf16: broadcast-AP product tensors + halving
    tree reductions; exp on ACT; AV the same with a rearranged V copy
  - out-proj via constant block-diag Wo matmul; residual via identity
    accumulate-matmul of fp32 x; bo bias rides the ACT PSUM->SBUF copy
  - big batched DMAs out
"""

import json

import numpy as np
import ml_dtypes

import concourse.bass as bass
import concourse.mybir as mybir
from concourse.tile import TileContext
from concourse.bass_utils import run_bass_kernel_spmd
import concourse.bass_utils as _bass_utils
import concourse.bass2jax as _bass2jax
import bass_rust

F32 = mybir.dt.float32
BF16 = mybir.dt.bfloat16
AX = mybir.AxisListType
OP = mybir.AluOpType
AF = mybir.ActivationFunctionType

N_HEADS = 8
D = 64
A = 4          # attention groups
SD = 16        # sub dim per group
SCALE = SD ** -0.5
LN_EPS = 1e-5
N_CORES = 8

_PATCHED = False

# this walrus build accepts fewer sync-wait commands per instruction than
# bass emits; hoist the excess onto EventSemaphore carriers just before.
_WAIT_CAPS = {"Drain": 0, "Nop": 0, "EventSemaphore": 2}
_DEFAULT_WAIT_CAP = 1


def _fix_bir_waits(bir: bytes) -> bytes:
    j = json.loads(bir)
    ctr = 0
    changed = False
    for f in j.get("functions", []):
        for blk in f.get("blocks", []):
            out = []
            for ins in blk.get("instructions", []):
                si = ins.get("sync_info") or {}
                ow = si.get("on_wait") or []
                cap = _WAIT_CAPS.get(ins.get("opcode"), _DEFAULT_WAIT_CAP)
                if len(ow) > cap:
                    changed = True
                    n_keep = cap
                    excess, keep = ow[: len(ow) - n_keep], ow[len(ow) - n_keep :]
                    for i in range(0, len(excess), 2):
                        ctr += 1
                        chunk = excess[i : i + 2]
                        w0 = chunk[0]
                        out.append({
                            "debug": ins.get("debug", 0),
                            "engine": ins.get("engine"),
                            "ins": [],
                            "outs": [],
                            "name": f"waitfix_{ctr}",
                            "opcode": "EventSemaphore",
                            "sync_info": {
                                "on_update": [{
                                    "ant_name": w0["ant_name"],
                                    "id": w0["id"],
                                    "sync_type": "semaphore",
                                    "update_mode": "sem-add-imm",
                                    "update_value": 0,
                                }],
                                "on_wait": chunk,
                            },
                        })
                    si = dict(si)
                    si["on_wait"] = keep
                    ins = dict(ins)
                    ins["sync_info"] = si
                out.append(ins)
            blk["instructions"] = out
    if not changed:
        return bir
    return json.dumps(j).encode()


_orig_compile_bir_kernel = _bass_utils.compile_bir_kernel


def _compile_bir_kernel_fixed(bir_json, tmpdir, neff_name="file.neff"):
    if isinstance(bir_json, str):
        bir_json = bir_json.encode()
    return _orig_compile_bir_kernel(_fix_bir_waits(bir_json), tmpdir, neff_name=neff_name)


def _patch_tile_drain():
    """walrus here rejects >2 sem waits on the Tile tail-drain; spread the
    waits over EventSemaphore carriers (<=2 waits each) instead."""
    global _PATCHED
    if _PATCHED:
        return
    _PATCHED = True
    _bass_utils.compile_bir_kernel = _compile_bir_kernel_fixed
    _bass2jax.compile_bir_kernel = _compile_bir_kernel_fixed
    ScopedClock = bass_rust.ScopedClock

    def patched(self, tick_clock, wait_clock):
        nc = self.nc
        sems = list(self.sems.allocated().values())
        if sems:
            carrier = nc.sync.sem_inc(sems[0], 0)
            wait_clock.add_sem_waits(
                carrier.ins, ScopedClock({None: tick_clock.global_clock})
            )
            si = carrier.ins.sync_info
            waits = list(si.on_wait) if si else []
            if len(waits) > 2:
                carrier.ins.sync_info = bass_rust.SyncInfo(
                    on_wait=waits[:2], on_update=list(si.on_update)
                )
                for i in range(2, len(waits), 2):
                    c2 = nc.sync.sem_inc(sems[0], 0)
                    si2 = c2.ins.sync_info
                    c2.ins.sync_info = bass_rust.SyncInfo(
                        on_wait=waits[i : i + 2],
                        on_update=list(si2.on_update) if si2 else [],
                    )
        nc.sync.drain()
        nc.all_engine_barrier()
        popped = nc._tile_sem_poison_stack.pop()
        assert popped is self._sem_poison
        nc.clear_and_free_semaphores(sems)
        nc.all_engine_barrier()

    TileContext._drain_and_barrier = patched


def build_nc(n_b: int, s_total: int, st_sites: int):
    """Build the per-core SPMD program.

    n_b: batches per core; s_total: sites per batch (H*W);
    st_sites: sites per super-tile (DMA granularity), multiple of 128.
    """
    _patch_tile_drain()
    nc = bass.Bass()
    TILE = 128
    n_st = s_total // st_sites
    n_t = st_sites // TILE
    NC4 = 4  # head-pair chunks

    x_d = nc.dram_tensor("x", [n_b, N_HEADS, D, s_total], F32, kind="ExternalInput")
    wq_d = nc.dram_tensor("wq_bd", [128, 128], BF16, kind="ExternalInput")
    wk_d = nc.dram_tensor("wk_bd", [128, 128], BF16, kind="ExternalInput")
    wv_d = nc.dram_tensor("wv_bd", [128, 128], BF16, kind="ExternalInput")
    wo_d = nc.dram_tensor("wo_bd", [128, 128], BF16, kind="ExternalInput")
    idf_d = nc.dram_tensor("ident_f32", [128, 128], F32, kind="ExternalInput")
    idb_d = nc.dram_tensor("ident_bf", [128, 128], BF16, kind="ExternalInput")
    ones_d = nc.dram_tensor("ones_row", [1, 128], BF16, kind="ExternalInput")
    bq_d = nc.dram_tensor("biasq_row", [1, 512], BF16, kind="ExternalInput")
    bk_d = nc.dram_tensor("biask_row", [1, 512], BF16, kind="ExternalInput")
    bv_d = nc.dram_tensor("biasv_row", [1, 512], BF16, kind="ExternalInput")
    bo_d = nc.dram_tensor("bo_col", [128, 1], F32, kind="ExternalInput")
    eps_d = nc.dram_tensor("eps_col", [128, 1], F32, kind="ExternalInput")
    out_d = nc.dram_tensor("out", [n_b, N_HEADS, D, s_total], F32, kind="ExternalOutput")

    with TileContext(nc) as tc:
        with (
            tc.tile_pool(name="consts", bufs=1) as cpool,
            tc.tile_pool(name="xio", bufs=2) as xpool,
            tc.tile_pool(name="oio", bufs=2) as opool,
            tc.tile_pool(name="work", bufs=3) as wpool,
            tc.tile_pool(name="vecs", bufs=4) as vpool,
            tc.tile_pool(name="stats", bufs=2) as spool,
            tc.tile_pool(name="ps", bufs=1, space="PSUM") as pspool,
            tc.tile_pool(name="psxs", bufs=2, space="PSUM") as xspool,
            tc.tile_pool(name="psqkv", bufs=1, space="PSUM") as qkvpool,
        ):
            # ---- constants into SBUF
            def cload(dram, shape, dtype, tag):
                t = cpool.tile(shape, dtype, tag=tag)
                nc.sync.dma_start(out=t[:], in_=dram[:])
                return t

            wq = cload(wq_d, [128, 128], BF16, "wq")
            wk = cload(wk_d, [128, 128], BF16, "wk")
            wv = cload(wv_d, [128, 128], BF16, "wv")
            wo = cload(wo_d, [128, 128], BF16, "wo")
            idf = cload(idf_d, [128, 128], F32, "idf")
            idb = cload(idb_d, [128, 128], BF16, "idb")
            ones_row = cload(ones_d, [1, 128], BF16, "ones")
            bq = cload(bq_d, [1, 512], BF16, "bq")
            bk = cload(bk_d, [1, 512], BF16, "bk")
            bv = cload(bv_d, [1, 512], BF16, "bv")
            bo = cload(bo_d, [128, 1], F32, "bo")
            eps = cload(eps_d, [128, 1], F32, "eps")

            for b in range(n_b):
                for st in range(n_st):
                    # ---- load super-tile: 4 chunks of [128=(2n,64d), st_sites]
                    x_sb = xpool.tile([128, NC4 * st_sites], F32, tag="x_sb")
                    xv = x_d[b].rearrange("n d s -> (n d) s")
                    for c in range(NC4):
                        nc.sync.dma_start(
                            out=x_sb[:, c * st_sites : (c + 1) * st_sites],
                            in_=xv[c * 128 : (c + 1) * 128,
                                   st * st_sites : (st + 1) * st_sites],
                        )
                    out_sb = opool.tile([128, NC4 * st_sites], F32, tag="out_sb")

                    # ---- phase A: LN stats for all tiles of this super-tile
                    sums = spool.tile([128, n_t * 8], F32, tag="sums")
                    sq = spool.tile([128, n_t * 8], F32, tag="sq")
                    for k in range(n_t):
                        ps_xs = xspool.tile([128, 512], F32, tag="ps_xs")
                        for c in range(NC4):
                            nc.tensor.transpose(
                                ps_xs[:, c * 128 : (c + 1) * 128],
                                x_sb[:, c * st_sites + k * TILE :
                                        c * st_sites + (k + 1) * TILE],
                                idf[:],
                            )
                        xsq = vpool.tile([128, 512], BF16, tag="xsq")
                        nc.scalar.activation(xsq[:], ps_xs[:], AF.Square)
                        nc.vector.tensor_reduce(
                            sums[:, k * 8 : (k + 1) * 8],
                            ps_xs[:].rearrange("p (g d) -> p g d", g=8),
                            axis=AX.X, op=OP.add,
                        )
                        nc.vector.tensor_reduce(
                            sq[:, k * 8 : (k + 1) * 8],
                            xsq[:].rearrange("p (g d) -> p g d", g=8),
                            axis=AX.X, op=OP.add,
                        )
                    # batched stat math over [128, (k, g)]
                    mu = spool.tile([128, n_t * 8], F32, tag="mu")
                    var = spool.tile([128, n_t * 8], F32, tag="var")
                    rstd = spool.tile([128, n_t * 8], F32, tag="rstd")
                    mb = spool.tile([128, n_t * 8], F32, tag="mb")
                    nc.vector.tensor_scalar(mu[:], sums[:], 1.0 / 64.0, None, op0=OP.mult)
                    nc.vector.tensor_tensor(var[:], mu[:], mu[:], op=OP.mult)
                    # var = sq/64 - mu^2
                    nc.vector.scalar_tensor_tensor(
                        var[:], sq[:], 1.0 / 64.0, var[:], op0=OP.mult, op1=OP.subtract
                    )
                    # std = sqrt(var + eps) ; rstd = 1/std ; mb = -mu*rstd
                    nc.scalar.activation(var[:], var[:], AF.Sqrt, bias=eps[:, 0:1])
                    nc.vector.reciprocal(rstd[:], var[:])
                    nc.vector.scalar_tensor_tensor(
                        mb[:], mu[:], -1.0, rstd[:], op0=OP.mult, op1=OP.mult
                    )
                    rstd3 = rstd.rearrange("p (k g) -> p k g", k=n_t)
                    mb3 = mb.rearrange("p (k g) -> p k g", k=n_t)

                    # ---- phase B: per tile
                    for k in range(n_t):
                        ps_xs = xspool.tile([128, 512], F32, tag="ps_xs")
                        for c in range(NC4):
                            nc.tensor.transpose(
                                ps_xs[:, c * 128 : (c + 1) * 128],
                                x_sb[:, c * st_sites + k * TILE :
                                        c * st_sites + (k + 1) * TILE],
                                idf[:],
                            )
                        # xhat = x*rstd + (-mu*rstd), bf16
                        xhat = vpool.tile([128, 512], BF16, tag="xhat")
                        x3 = ps_xs[:].rearrange("p (g d) -> p g d", g=8)
                        xh3 = xhat[:].rearrange("p (g d) -> p g d", g=8)
                        nc.vector.tensor_tensor(
                            xh3, x3,
                            rstd3[:, k].unsqueeze(2).broadcast_to([128, 8, 64]),
                            op=OP.mult,
                        )
                        nc.vector.tensor_tensor(
                            xh3, xh3,
                            mb3[:, k].unsqueeze(2).broadcast_to([128, 8, 64]),
                            op=OP.add,
                        )
                        ps_xhT = pspool.tile([128, 512], BF16, tag="ps_xhT")
                        for c in range(NC4):
                            nc.tensor.transpose(
                                ps_xhT[:, c * 128 : (c + 1) * 128],
                                xhat[:, c * 128 : (c + 1) * 128],
                                idb[:],
                            )
                        xhT = vpool.tile([128, 512], BF16, tag="xhT")
                        nc.scalar.copy(xhT[:], ps_xhT[:])

                        ps_q = qkvpool.tile([128, 512], F32, tag="ps_q")
                        ps_k = qkvpool.tile([128, 512], F32, tag="ps_k")
                        ps_v = qkvpool.tile([128, 512], F32, tag="ps_v")
                        # bias first: full-bank start=True sets all bits;
                        # then per chunk all 3 projections share the stationary
                        for ps_p, b_p in ((ps_q, bq), (ps_k, bk), (ps_v, bv)):
                            nc.tensor.matmul(
                                ps_p[:], ones_row[:], b_p[:], start=True, stop=False
                            )
                        for c in range(NC4):
                            for ps_p, w_p in ((ps_q, wq), (ps_k, wk), (ps_v, wv)):
                                nc.tensor.matmul(
                                    ps_p[:, c * 128 : (c + 1) * 128],
                                    xhT[:, c * 128 : (c + 1) * 128],
                                    w_p[:],
                                    start=False,
                                    stop=(c == NC4 - 1),
                                )
                        q_sb = vpool.tile([128, 512], BF16, tag="q_sb")
                        k_sb = vpool.tile([128, 512], BF16, tag="k_sb")
                        v_sb = vpool.tile([128, 512], BF16, tag="v_sb")
                        nc.scalar.copy(q_sb[:], ps_q[:])
                        nc.scalar.copy(k_sb[:], ps_k[:])
                        # V stored (a, t, j) so AV products get unit inner strides
                        nc.scalar.activation(
                            v_sb[:].rearrange("p (a t j) -> p a j t", a=A, t=SD, j=8),
                            ps_v[:].rearrange("p (j a t) -> p a j t", j=8, a=A, t=SD),
                            AF.Copy,
                        )

                        # scores products [128, (a,i,j,t)] — split per a
                        # (walrus TT supports at most 3 free dims per AP)
                        prod_s = wpool.tile([128, 4096], BF16, tag="bigs")
                        qv = q_sb[:].rearrange("p (i a t) -> p a i t", i=8, a=A, t=SD)
                        kv = k_sb[:].rearrange("p (j a t) -> p a j t", j=8, a=A, t=SD)
                        pv5 = prod_s[:].rearrange(
                            "p (a i j t) -> p a i j t", a=A, i=8, j=8, t=SD
                        )
                        for a in range(A):
                            eng_s = nc.gpsimd if a == 3 else nc.vector
                            eng_s.tensor_tensor(
                                pv5[:, a],
                                qv[:, a].unsqueeze(2).broadcast_to([128, 8, 8, SD]),
                                kv[:, a].unsqueeze(1).broadcast_to([128, 8, 8, SD]),
                                op=OP.mult,
                            )
                        # t-tree: 16 -> 8 -> 4 -> 2 -> 1
                        t8 = wpool.tile([128, 2048], BF16, tag="t4k")
                        pv = prod_s[:].rearrange("p (r t) -> p r t", r=256, t=SD)
                        nc.vector.tensor_tensor(
                            t8[:].rearrange("p (r t) -> p r t", r=256, t=8),
                            pv[:, :, 0:8], pv[:, :, 8:16], op=OP.add,
                        )
                        t4 = wpool.tile([128, 1024], BF16, tag="t2k")
                        t8v = t8[:].rearrange("p (r t) -> p r t", r=256, t=8)
                        nc.vector.tensor_tensor(
                            t4[:].rearrange("p (r t) -> p r t", r=256, t=4),
                            t8v[:, :, 0:4], t8v[:, :, 4:8], op=OP.add,
                        )
                        t2 = wpool.tile([128, 512], BF16, tag="t1k")
                        t4v = t4[:].rearrange("p (r t) -> p r t", r=256, t=4)
                        nc.vector.tensor_tensor(
                            t2[:].rearrange("p (r t) -> p r t", r=256, t=2),
                            t4v[:, :, 0:2], t4v[:, :, 2:4], op=OP.add,
                        )
                        scores = wpool.tile([128, 256], BF16, tag="sc")
                        t2v = t2[:].rearrange("p (r t) -> p r t", r=256, t=2)
                        nc.vector.tensor_tensor(
                            scores[:].rearrange("p (r t) -> p r t", r=256, t=1),
                            t2v[:, :, 0:1], t2v[:, :, 1:2], op=OP.add,
                        )
                        e = wpool.tile([128, 256], BF16, tag="e")
                        nc.scalar.activation(e[:], scores[:], AF.Exp, scale=SCALE)
                        den = wpool.tile([128, 32], F32, tag="den")
                        nc.vector.tensor_reduce(
                            den[:],
                            e[:].rearrange("p (r j) -> p r j", r=32, j=8),
                            axis=AX.X, op=OP.add,
                        )
                        rcp = wpool.tile([128, 32], BF16, tag="rcp")
                        with nc.allow_low_precision(reason="softmax denom in bf16"):
                            nc.vector.reciprocal(rcp[:], den[:])

                        # AV products [128, (a,i,t,j)] — split per a
                        prod_av = wpool.tile([128, 4096], BF16, tag="bigav")
                        ev = e[:].rearrange("p (a i j) -> p a i j", a=A, i=8, j=8)
                        vv = v_sb[:].rearrange("p (a t j) -> p a t j", a=A, t=SD, j=8)
                        av5 = prod_av[:].rearrange(
                            "p (a i t j) -> p a i t j", a=A, i=8, t=SD, j=8
                        )
                        for a in range(A):
                            eng = nc.vector if a == 0 else nc.gpsimd
                            eng.tensor_tensor(
                                av5[:, a],
                                ev[:, a].unsqueeze(2).broadcast_to([128, 8, SD, 8]),
                                vv[:, a].unsqueeze(1).broadcast_to([128, 8, SD, 8]),
                                op=OP.mult,
                            )
                        # j-tree: 8 -> 4 -> 2 -> 1
                        j4 = wpool.tile([128, 2048], BF16, tag="t4k")
                        av = prod_av[:].rearrange("p (r j) -> p r j", r=512, j=8)
                        nc.vector.tensor_tensor(
                            j4[:].rearrange("p (r j) -> p r j", r=512, j=4),
                            av[:, :, 0:4], av[:, :, 4:8], op=OP.add,
                        )
                        j2 = wpool.tile([128, 1024], BF16, tag="t2k")
                        j4v = j4[:].rearrange("p (r j) -> p r j", r=512, j=4)
                        nc.vector.tensor_tensor(
                            j2[:].rearrange("p (r j) -> p r j", r=512, j=2),
                            j4v[:, :, 0:2], j4v[:, :, 2:4], op=OP.add,
                        )
                        o_un = wpool.tile([128, 512], BF16, tag="t1k")
                        j2v = j2[:].rearrange("p (r j) -> p r j", r=512, j=2)
                        nc.vector.tensor_tensor(
                            o_un[:].rearrange("p (r j) -> p r j", r=512, j=1),
                            j2v[:, :, 0:1], j2v[:, :, 1:2], op=OP.add,
                        )
                        # normalize by 1/den and reorder (a,i,t) -> (i,a,t)=(n,d)
                        attnout = vpool.tile([128, 512], BF16, tag="attnout")
                        rb = (
                            rcp[:]
                            .rearrange("p (a i) -> p a i", a=A, i=8)
                            .unsqueeze(3)
                            .broadcast_to([128, A, 8, SD])
                        )
                        nc.vector.tensor_tensor(
                            attnout[:].rearrange(
                                "p (i a t) -> p a i t", i=8, a=A, t=SD
                            ),
                            o_un[:].rearrange("p (a i t) -> p a i t", a=A, i=8, t=SD),
                            rb, op=OP.mult,
                        )
                        # out projection
                        ps_aoT = pspool.tile([128, 512], BF16, tag="ps_aoT")
                        for c in range(NC4):
                            nc.tensor.transpose(
                                ps_aoT[:, c * 128 : (c + 1) * 128],
                                attnout[:, c * 128 : (c + 1) * 128],
                                idb[:],
                            )
                        ao_sb = vpool.tile([128, 512], BF16, tag="ao_sb")
                        nc.scalar.copy(ao_sb[:], ps_aoT[:])
                        ps_o = pspool.tile([128, 512], F32, tag="ps_o")
                        # residual first (c0 start=True clears bank)
                        for c in range(NC4):
                            nc.tensor.matmul(
                                ps_o[:, c * 128 : (c + 1) * 128],
                                idf[:],
                                x_sb[:, c * st_sites + k * TILE :
                                        c * st_sites + (k + 1) * TILE],
                                start=(c == 0), stop=False,
                            )
                        for c in range(NC4):
                            nc.tensor.matmul(
                                ps_o[:, c * 128 : (c + 1) * 128],
                                wo[:],
                                ao_sb[:, c * 128 : (c + 1) * 128],
                                start=False, stop=(c == NC4 - 1),
                            )
                        # + bo, PSUM -> out_sb
                        ov = out_sb[:].rearrange(
                            "p (c s) -> p c s", c=NC4, s=st_sites
                        )[:, :, k * TILE : (k + 1) * TILE]
                        nc.scalar.activation(
                            ov,
                            ps_o[:].rearrange("p (c s) -> p c s", c=NC4, s=TILE),
                            AF.Identity,
                            bias=bo[:, 0:1],
                        )
                    # ---- store super-tile
                    ovd = out_d[b].rearrange("n d s -> (n d) s")
                    for c in range(NC4):
                        nc.sync.dma_start(
                            out=ovd[c * 128 : (c + 1) * 128,
                                    st * st_sites : (st + 1) * st_sites],
                            in_=out_sb[:, c * st_sites : (c + 1) * st_sites],
                        )
    return nc


def _prep_consts(Wq, bq, Wk, bk, Wv, bv, Wo, bo, ln_g, ln_b):
    f32 = np.float32
    bf = ml_dtypes.bfloat16
    Wq, bq, Wk, bk, Wv, bv, Wo, bo, ln_g, ln_b = [
        np.asarray(t, f32) for t in (Wq, bq, Wk, bk, Wv, bv, Wo, bo, ln_g, ln_b)
    ]
    # fold LN affine into projections: xn = xhat*g + b
    # y = xn @ W.T + bias = xhat @ (W*g).T + (W @ b + bias)
    Wq_p, bq_p = Wq * ln_g[None, :], bq + Wq @ ln_b
    Wk_p, bk_p = Wk * ln_g[None, :], bk + Wk @ ln_b
    Wv_p, bv_p = Wv * ln_g[None, :], bv + Wv @ ln_b

    def blockdiag(W):
        # bd[h*64+d, h*64+o] = W[o, d]
        m = np.zeros((128, 128), f32)
        m[:64, :64] = W.T
        m[64:, 64:] = W.T
        return m.astype(bf)

    consts = {
        "wq_bd": blockdiag(Wq_p),
        "wk_bd": blockdiag(Wk_p),
        "wv_bd": blockdiag(Wv_p),
        "wo_bd": blockdiag(Wo),
        "ident_f32": np.eye(128, dtype=f32),
        "ident_bf": np.eye(128).astype(bf),
        "ones_row": np.ones((1, 128), bf),
        "biasq_row": np.tile(bq_p, 8)[None, :].astype(bf),
        "biask_row": np.tile(bk_p, 8)[None, :].astype(bf),
        "biasv_row": np.tile(bv_p, 8)[None, :].astype(bf),
        "bo_col": np.tile(bo, 2)[:, None].astype(f32),
        "eps_col": np.full((128, 1), LN_EPS, f32),
    }
    return consts


def kernel(x, Wq, bq, Wk, bk, Wv, bv, Wo, bo, ln_g, ln_b):
    x = np.asarray(x, np.float32)
    B, n, d, H, W = x.shape
    S = H * W
    bpc = B // N_CORES
    consts = _prep_consts(Wq, bq, Wk, bk, Wv, bv, Wo, bo, ln_g, ln_b)

    nc = build_nc(n_b=bpc, s_total=S, st_sites=1024 if S % 1024 == 0 else S)
    xr = x.reshape(B, n, d, S)
    in_maps = []
    for c in range(N_CORES):
        m = dict(consts)
        m["x"] = np.ascontiguousarray(xr[c * bpc : (c + 1) * bpc])
        in_maps.append(m)
    res = run_bass_kernel_spmd(nc, in_maps, core_ids=list(range(N_CORES)))
    outs = [res.results[i]["out"] for i in range(N_CORES)]
    out = np.concatenate(outs, axis=0).reshape(B, n, d, H, W)
    return out.astype(np.float32)

